# revision 1
# baseline (speedup 1.0000x reference)
"""Trainium2 Bass kernel for nn_CustomLoss_40097814676083.

Math: per sample i with logits o[i, :C], target t, age-derived (delta, shift):
    soft = (1-delta)*onehot(t) + delta*onehot((t+shift) % C)
    loss_i = logsumexp(o_i) - [(1-delta)*o[i,t] + delta*o[i,n]]
    out = mean_i loss_i
(sum of soft-target weights is exactly 1, so the logsumexp term has unit
coefficient; |o| <= ~6 for randn inputs so exp without max-subtraction is
safe in f32.)

Device strategy (pure data parallel over 8 cores, B_core = 262144):
  layout [128 partitions x 2048 samples], each sample's 18 classes contiguous.
  - ScalarE: E = exp(O), then lse = ln(s) with per-partition accumulation.
  - VectorE: s = segmented reduce_sum over the 18-class axis (3D AP).
  - GPSIMD:  local_scatter builds W = (1-delta)@t + delta@n scaled one-hot
             rows (fp16) from host-precomputed indices/values.
  - TensorE: the gather term only needs a TOTAL sum, which equals
             trace(O^T W) = sum of diagonals of small block matmuls;
             accumulate all blocks into one PSUM [128,126] tile
             (lhsT = O16 fp16, rhs = W fp16; lhsT uses 128-column blocks
             to trigger fast weight load).
  - O -> fp16 conversion is split between ScalarE and VectorE so neither
    exceeds the DMA roofline.
  Host: loss = (sum(lse) - trace) / B.
"""

import numpy as np

B = 2097152
C = 18
NCORES = 8
BC = B // NCORES          # 262144 samples per core
P = 128                   # partitions
GP = BC // P              # 2048 samples per partition row
SWIN = 64                 # groups per local_scatter window
NWIN = GP // SWIN         # 32 windows per core
WELEMS = SWIN * C         # 1152 elements per scatter window
BLK = 7                   # groups per trace-matmul block (126 columns)
MCOL = BLK * C            # 126
TILES = [256] * 8
assert sum(TILES) == GP and all(t % SWIN == 0 for t in TILES)
# gather-term implementation: "f16" = fp16 W scatter + fp16 O copy;
# "pair" = scatter f32 bit-halves as uint16 pairs, matmul f32r x f32r on
# bitcast views (no O conversion pass at all)
GATHER = "f16"
SWIN2 = 32                # groups per pair-scatter window
NWIN2 = GP // SWIN2       # 64 windows per core
WELEMS2 = SWIN2 * C * 2   # 1152 uint16 elements per pair window

_CACHE = {}


def _build_bass(repeats=1, mode="full", conv=None, gather=None, fold=True):
    from contextlib import ExitStack

    import concourse.bacc as bacc
    import concourse.tile as tile
    from concourse import library_config, mybir

    if gather is None:
        gather = GATHER
    nc = bacc.Bacc("TRN2", debug=False)
    o = nc.dram_tensor("o", [BC, C], mybir.dt.float32, kind="ExternalInput").ap()
    if gather == "pair":
        meta = nc.dram_tensor(
            "meta", [P, NWIN2, 2, 4 * SWIN2], mybir.dt.uint16, kind="ExternalInput"
        ).ap()
    else:
        meta = nc.dram_tensor(
            "meta", [P, NWIN, 2, 2 * SWIN], mybir.dt.uint16, kind="ExternalInput"
        ).ap()
    lse_out = nc.dram_tensor(
        "lse_out", [P, 1], mybir.dt.float32, kind="ExternalOutput"
    ).ap()
    TRN = 256 if gather == "pair" else MCOL
    TRM = MCOL if gather == "pair" else P
    tr_out = nc.dram_tensor(
        "tr_out", [TRM, TRN], mybir.dt.float32, kind="ExternalOutput"
    ).ap()

    o_v = o.rearrange("(p n) c -> p n c", p=P)  # [128, 2048, 18]

    Exp = mybir.ActivationFunctionType.Exp
    Ln = mybir.ActivationFunctionType.Ln
    X = mybir.AxisListType.X
    f32 = mybir.dt.float32
    f16 = mybir.dt.float16
    i16 = mybir.dt.int16

    if conv is None:
        # which engine converts each tile's f32 logits to fp16:
        # a=ScalarE, v=VectorE, p=GPSIMD, h=half ScalarE/half VectorE
        conv = "avvvvavv"
    with tile.TileContext(nc) as tc, ExitStack() as ctx:
        nc.gpsimd.load_library(library_config.local_scatter)

        bufs = globals().get("POOL_BUFS", {})
        opool = ctx.enter_context(tc.tile_pool(name="opool", bufs=bufs.get("opool", 4)))
        epool = ctx.enter_context(tc.tile_pool(name="epool", bufs=bufs.get("epool", 2)))
        hpool = ctx.enter_context(tc.tile_pool(name="hpool", bufs=bufs.get("hpool", 2)))
        wpool = ctx.enter_context(tc.tile_pool(name="wpool", bufs=bufs.get("wpool", 3)))
        o16pool = ctx.enter_context(tc.tile_pool(name="o16pool", bufs=bufs.get("o16pool", 3)))
        singles = ctx.enter_context(tc.tile_pool(name="singles", bufs=1))
        pspool = ctx.enter_context(tc.tile_pool(name="ps", bufs=1, space="PSUM"))

        if gather == "pair":
            meta_sb = singles.tile([P, NWIN2, 2, 4 * SWIN2], mybir.dt.uint16)
            sdat_sb = meta_sb[:, :, 0, :]
            sidx_sb = meta_sb[:, :, 1, :].bitcast(i16)
            META_CHUNK = 16  # pair windows per meta DMA chunk
            NCHUNK = NWIN2 // META_CHUNK
        else:
            meta_sb = singles.tile([P, NWIN, 2, 2 * SWIN], mybir.dt.uint16)
            sdat_sb = meta_sb[:, :, 0, :].bitcast(f16)
            sidx_sb = meta_sb[:, :, 1, :].bitcast(i16)
            META_CHUNK = 8  # windows per meta DMA chunk
            NCHUNK = NWIN // META_CHUNK
        lse_sb = singles.tile([P, 1], f32)
        s_all = singles.tile([P, GP], f32)
        psum_tr = pspool.tile([MCOL, TRN] if gather == "pair" else [P, MCOL], f32)

        max_tile = max(TILES)
        for rep in range(repeats):
          g0 = 0
          for ti, gt in enumerate(TILES):
              free_t = gt * C
              ot = opool.tile([P, max_tile, C], f32, tag="ot", name=f"ot{ti}")[:, :gt, :]
              nc.sync.dma_start(out=ot, in_=o_v[:, g0 : g0 + gt, :])
              if ti < NCHUNK:
                  mlo = ti * META_CHUNK
                  nc.sync.dma_start(
                      out=meta_sb[:, mlo : mlo + META_CHUNK, :, :],
                      in_=meta[:, mlo : mlo + META_CHUNK, :, :],
                  )

              if mode != "dma" and not fold:
                  etb = epool.tile(
                      [P, max_tile, C], mybir.dt.bfloat16, tag="et", name=f"etb{ti}"
                  )[:, :gt, :]
                  nc.scalar.activation(etb, ot, Exp)
                  nc.vector.reduce_sum(s_all[:, g0 : g0 + gt], etb, axis=X)
              elif mode != "dma":
                  # split-class fold: exp writes class halves 0-8 / 9-17 into
                  # two contiguous fp16 buffers (two 3D-AP instructions); one
                  # step-1 fp16 tensor_add folds them at the DVE 2x_1P rate so
                  # the 1x-rate segmented reduce reads only 9 classes/sample.
                  et = epool.tile(
                      [P, 2, max_tile * 9], f16, tag="et", name=f"et{ti}"
                  )[:, :, : gt * 9]
                  for hh in range(2):
                      nc.scalar.activation(
                          et[:, hh, :].rearrange("p (g k) -> p g k", k=9),
                          ot[:, :, 9 * hh : 9 * hh + 9],
                          Exp,
                      )
                  ht = hpool.tile(
                      [P, max_tile * 9], f16, tag="ht", name=f"ht{ti}"
                  )[:, : gt * 9]
                  nc.vector.tensor_add(ht, et[:, 0, :], et[:, 1, :])
                  nc.vector.reduce_sum(
                      s_all[:, g0 : g0 + gt],
                      ht.rearrange("p (g k) -> p g k", k=9),
                      axis=X,
                  )
              if mode != "full":
                  g0 += gt
                  continue
              of = ot.rearrange("p n c -> p (n c)")
              n_blocks = free_t // MCOL
              rem = free_t - n_blocks * MCOL
              # remainder block (if any) second so the first/last matmuls span
              # the full psum partition range (sim group tracking requires
              # start/stop to cover every started zero region)
              order = [0] + ([n_blocks] if rem else []) + list(range(1, n_blocks))

              if gather == "pair":
                  # order: narrow blocks (remainder, final 198-wide) go in the
                  # middle so first/last matmuls span the full [126, 256] psum
                  # zero-region set
                  order = (
                      [0]
                      + ([n_blocks] if rem else [])
                      + [n_blocks - 1]
                      + list(range(1, n_blocks - 1))
                  )
                  # W holds raw f32 bit patterns, scattered as uint16 pairs
                  wtu = wpool.tile(
                      [P, max_tile * C * 2], mybir.dt.uint16, tag="wt",
                      name=f"wt{ti}",
                  )[:, : free_t * 2]
                  w0 = g0 // SWIN2
                  for w in range(gt // SWIN2):
                      nc.gpsimd.local_scatter(
                          wtu[:, w * WELEMS2 : (w + 1) * WELEMS2],
                          sdat_sb[:, w0 + w, :],
                          sidx_sb[:, w0 + w, :],
                          channels=P,
                          num_elems=WELEMS2,
                          num_idxs=4 * SWIN2,
                      )
                  wr = wtu.bitcast(mybir.dt.float32r)
                  orr = of.bitcast(mybir.dt.float32r)
                  for pos, j in enumerate(order):
                      lo = j * MCOL
                      if j < n_blocks:
                          nr = min(TRN, free_t - lo)
                          nc.tensor.matmul(
                              psum_tr[:MCOL, :nr],
                              wr[:, lo : lo + MCOL],
                              orr[:, lo : lo + nr],
                              start=(ti == 0 and pos == 0),
                              stop=(ti == len(TILES) - 1 and pos == len(order) - 1),
                          )
                      else:
                          nc.tensor.matmul(
                              psum_tr[:rem, :rem],
                              wr[:, lo : lo + rem],
                              orr[:, lo : lo + rem],
                              start=False,
                              stop=False,
                          )
                  g0 += gt
                  continue

              wt = wpool.tile([P, max_tile * C], f16, tag="wt", name=f"wt{ti}")[:, :free_t]
              w0 = g0 // SWIN
              for w in range(gt // SWIN):
                  nc.gpsimd.local_scatter(
                      wt[:, w * WELEMS : (w + 1) * WELEMS],
                      sdat_sb[:, w0 + w, :],
                      sidx_sb[:, w0 + w, :],
                      channels=P,
                      num_elems=WELEMS,
                      num_idxs=2 * SWIN,
                  )

              o16 = o16pool.tile([P, max_tile * C], f16, tag="o16", name=f"o16_{ti}")[:, :free_t]
              if conv[ti] == "a":
                  nc.scalar.copy(o16, of)
              elif conv[ti] == "p":
                  nc.gpsimd.tensor_copy(o16, of)
              elif conv[ti] == "h":
                  half = free_t // 2
                  nc.scalar.copy(o16[:, :half], of[:, :half])
                  nc.vector.tensor_copy(o16[:, half:], of[:, half:])
              else:
                  nc.vector.tensor_copy(o16, of)

              for pos, j in enumerate(order):
                  lo = j * MCOL
                  if j < n_blocks:
                      # 128-column lhsT (spills into the next block) enables
                      # fast weight load; extra output rows land in unused
                      # psum partitions.
                      ml = min(P, free_t - lo)
                      nc.tensor.matmul(
                          psum_tr[:ml, :MCOL],
                          o16[:, lo : lo + ml],
                          wt[:, lo : lo + MCOL],
                          start=(ti == 0 and pos == 0),
                          stop=(ti == len(TILES) - 1 and pos == len(order) - 1),
                      )
                  else:
                      nc.tensor.matmul(
                          psum_tr[:rem, :rem],
                          o16[:, lo : lo + rem],
                          wt[:, lo : lo + rem],
                          start=False,
                          stop=False,
                      )
              g0 += gt

        if mode == "dma":
            nc.vector.memset(s_all, 1.0)
        nc.scalar.activation(s_all, s_all, Ln, accum_out=lse_sb)

        tr_sb = singles.tile([TRM, TRN], f32)
        if mode == "full":
            nc.vector.tensor_copy(tr_sb, psum_tr)
        else:
            nc.vector.memset(tr_sb, 0.0)
        nc.sync.dma_start(out=tr_out, in_=tr_sb)
        nc.sync.dma_start(out=lse_out, in_=lse_sb)

    nc.compile()
    return nc


def _host_prep(outputs, targets, ages):
    """Shard outputs and build per-core scatter index/value tables."""
    t = np.asarray(targets).astype(np.int32)
    age = np.asarray(ages).astype(np.int32)

    b1 = (age > 50) & (age < 60)
    b2 = age == 60
    b3 = (age > 24) & (age < 30)
    b4 = (age > 29) & (age < 35)
    agef = age.astype(np.float32)
    delta = np.where(
        b1,
        (agef - 50) * np.float32(0.05),
        np.where(
            b2,
            np.float32(0.2),
            np.where(
                b3,
                (agef - 20) * np.float32(0.05),
                np.where(b4, (np.float32(39) - agef) * np.float32(0.05), np.float32(0)),
            ),
        ),
    ).astype(np.float32)
    shift = np.where(b1 | b3, 1, np.where(b2 | b4, -1, 0)).astype(np.int32)
    neigh = (t + shift) % C

    a16 = (np.float32(1.0) - delta).astype(np.float16)
    b16 = delta.astype(np.float16)

    if GATHER == "pair":
        # scatter f32 bit halves: per window of SWIN2 samples, 4 entries per
        # sample (a_lo, a_hi, b_lo, b_hi) at uint16 positions 2*(j*C+cls)+{0,1}
        a32 = (np.float32(1.0) - delta).view(np.uint32)
        b32 = delta.view(np.uint32)
        jj2 = (np.arange(SWIN2, dtype=np.int16) * C)[None, None, None, :]
        t_r = t.reshape(NCORES, P, NWIN2, SWIN2).astype(np.int16)
        n_r = neigh.reshape(NCORES, P, NWIN2, SWIN2).astype(np.int16)
        d_r = delta.reshape(NCORES, P, NWIN2, SWIN2)
        a_lo = (a32 & 0xFFFF).astype(np.uint16).reshape(NCORES, P, NWIN2, SWIN2)
        a_hi = (a32 >> 16).astype(np.uint16).reshape(NCORES, P, NWIN2, SWIN2)
        b_lo = (b32 & 0xFFFF).astype(np.uint16).reshape(NCORES, P, NWIN2, SWIN2)
        b_hi = (b32 >> 16).astype(np.uint16).reshape(NCORES, P, NWIN2, SWIN2)
        S = SWIN2
        meta = np.empty((NCORES, P, NWIN2, 2, 4 * S), np.uint16)
        dat = meta[..., 0, :]
        dat[..., 0 * S : 1 * S] = a_lo
        dat[..., 1 * S : 2 * S] = a_hi
        dat[..., 2 * S : 3 * S] = b_lo
        dat[..., 3 * S : 4 * S] = b_hi
        idx = meta[..., 1, :].view(np.int16)
        base_t = (jj2 + t_r).astype(np.int16) * np.int16(2)
        base_n = (jj2 + n_r).astype(np.int16) * np.int16(2)
        neg = np.int16(-1)
        idx[..., 0 * S : 1 * S] = base_t
        idx[..., 1 * S : 2 * S] = base_t + np.int16(1)
        idx[..., 2 * S : 3 * S] = np.where(d_r == 0, neg, base_n)
        idx[..., 3 * S : 4 * S] = np.where(d_r == 0, neg, base_n + np.int16(1))
    else:
        # sample s = core*BC + p*GP + win*SWIN + j
        jj = (np.arange(SWIN, dtype=np.int16) * C)[None, None, None, :]
        t_r = t.reshape(NCORES, P, NWIN, SWIN).astype(np.int16)
        n_r = neigh.reshape(NCORES, P, NWIN, SWIN).astype(np.int16)
        d_r = delta.reshape(NCORES, P, NWIN, SWIN)

        # meta[..., 0, :] = fp16 scatter values, meta[..., 1, :] = int16 indices
        meta = np.empty((NCORES, P, NWIN, 2, 2 * SWIN), np.uint16)
        dat = meta[..., 0, :].view(np.float16)
        dat[..., :SWIN] = a16.reshape(NCORES, P, NWIN, SWIN)
        dat[..., SWIN:] = b16.reshape(NCORES, P, NWIN, SWIN)
        idx = meta[..., 1, :].view(np.int16)
        idx[..., :SWIN] = jj + t_r
        idx[..., SWIN:] = np.where(d_r == 0, np.int16(-1), jj + n_r)

    o_sh = np.ascontiguousarray(outputs, dtype=np.float32).reshape(NCORES, BC, C)

    in_maps = [{"o": o_sh[i], "meta": meta[i]} for i in range(NCORES)]
    return in_maps


def kernel(outputs, targets, ages):
    import os

    # NTFF tracing needs an axon profile hook this container lacks; make sure
    # a stray BASS_TRACE can't divert run_bass_kernel_spmd onto that path.
    os.environ["BASS_NEVER_TRACE"] = "1"
    from concourse import bass_utils

    if "nc" not in _CACHE:
        _CACHE["nc"] = _build_bass()
    nc = _CACHE["nc"]

    in_maps = _host_prep(outputs, targets, ages)
    res = bass_utils.run_bass_kernel_spmd(
        nc, in_maps, core_ids=list(range(NCORES))
    )

    total = np.float64(0.0)
    for r in res.results:
        total += r["lse_out"].astype(np.float64).sum()
        total -= np.trace(r["tr_out"][:MCOL, :MCOL].astype(np.float64))
    return np.float32(total / B)



# revision 17
# speedup vs baseline: 1.0435x; 1.0435x over previous
"""Trainium2 Bass kernel for nn_CustomLoss_40097814676083.

Math: per sample i with logits o[i, :C], target t, age-derived (delta, shift):
    soft = (1-delta)*onehot(t) + delta*onehot((t+shift) % C)
    loss_i = logsumexp(o_i) - [(1-delta)*o[i,t] + delta*o[i,n]]
    out = mean_i loss_i
(sum of soft-target weights is exactly 1, so the logsumexp term has unit
coefficient; |o| <= ~6 for randn inputs so exp without max-subtraction is
safe in f32.)

Device strategy (pure data parallel over 8 cores, B_core = 262144):
  layout [128 partitions x 2048 samples], each sample's 18 classes contiguous.
  - ScalarE: E = exp(O); per-tile lse accumulation via Ln + accum_out.
  - VectorE: s = segmented reduce_sum over the 18-class axis.
  - GPSIMD:  local_scatter builds W = (1-delta)@t + delta@n scaled one-hot
             rows (fp16) from host-precomputed indices/values.
  - TensorE: the gather term only needs a TOTAL sum, which equals
             trace(O^T W) = sum of diagonals of small block matmuls;
             accumulate all blocks into one PSUM [128,126] tile.
  - O -> fp16 conversion split between ScalarE and VectorE for balance.
  Tiles are sized small at both ends: a small first tile lets the scatter
  pipeline start early; small last tiles shrink the serial exp->reduce->Ln
  tail after the final (roofline-bound) input DMA completes.
  Host: loss = (sum(lse) - trace) / B.
"""

import numpy as np

B = 2097152
C = 18
NCORES = 8
BC = B // NCORES          # 262144 samples per core
P = 128                   # partitions
GP = BC // P              # 2048 samples per partition row
SWIN = 64                 # groups per local_scatter window
NWIN = GP // SWIN         # 32 windows per core
WELEMS = SWIN * C         # 1152 elements per scatter window
BLK = 7                   # groups per trace-matmul block (126 columns)
MCOL = BLK * C            # 126
TILES = [64, 256, 256, 256, 256, 256, 256, 256, 128, 64]
NT = len(TILES)
assert sum(TILES) == GP and all(t % SWIN == 0 for t in TILES)

_CACHE = {}


def _patch_act_tables():
    """Make every Exp/Ln activation resolve to the one table set that holds
    both functions, so the per-tile Exp/Ln interleave needs a single
    LoadActFuncSet instead of a reload on every switch. Entry names, order,
    and count are preserved (runtime set ids index the original list)."""
    if _CACHE.get("act_patched"):
        return
    import concourse.bacc as bacc
    from concourse import mybir

    orig = bacc.get_activation_tables

    def patched(arch):
        tabs = orig(arch)
        Exp = mybir.ActivationFunctionType.Exp
        Ln = mybir.ActivationFunctionType.Ln
        if any(Exp in v and Ln in v for v in tabs.values()):
            tabs = {
                k: (v if (Exp in v and Ln in v) else v - {Exp, Ln})
                for k, v in tabs.items()
            }
        return tabs

    bacc.get_activation_tables = patched
    _CACHE["act_patched"] = True


def _build_bass(repeats=1, mode="full", conv=None, tiles=None):
    from contextlib import ExitStack

    import concourse.bacc as bacc
    import concourse.tile as tile
    from concourse import library_config, mybir

    if tiles is None:
        tiles = TILES
    nt = len(tiles)
    _patch_act_tables()
    nc = bacc.Bacc("TRN2", debug=False)
    o = nc.dram_tensor("o", [BC, C], mybir.dt.float32, kind="ExternalInput").ap()
    meta = nc.dram_tensor(
        "meta", [P, NWIN, 2, 2 * SWIN], mybir.dt.uint16, kind="ExternalInput"
    ).ap()
    # per-sample softmax denominators stream out in f16; the cheap log+sum
    # runs on the host (0.7% extra DMA instead of a serial device Ln tail)
    s_out = nc.dram_tensor(
        "s_out", [P, GP], mybir.dt.float16, kind="ExternalOutput"
    ).ap()
    tr_out = nc.dram_tensor(
        "tr_out", [P, MCOL], mybir.dt.float32, kind="ExternalOutput"
    ).ap()

    o_v = o.rearrange("(p n) c -> p n c", p=P)  # [128, 2048, 18]

    Exp = mybir.ActivationFunctionType.Exp
    Ln = mybir.ActivationFunctionType.Ln
    X = mybir.AxisListType.X
    f32 = mybir.dt.float32
    f16 = mybir.dt.float16
    i16 = mybir.dt.int16

    if conv is None:
        # every tile's f32->fp16 logit conversion is split ~31%/69% between
        # ScalarE and VectorE: with exp on ScalarE and fold+reduce on
        # VectorE this equalizes both engines' per-tile work (~80% of the
        # DMA-paced tile budget each, no alternating overload spikes)
        conv = "t" * nt
    with tile.TileContext(nc) as tc, ExitStack() as ctx, nc.allow_low_precision(
        reason="f16 softmax denominators: |logits|<6 so s in [1e-3, 7e3]; "
        "f16 rounding of s adds ~5e-4 relative noise per sample which "
        "averages out over 2M samples (loss rel err ~1e-7)"
    ):
        nc.gpsimd.load_library(library_config.local_scatter)

        bufs = globals().get("POOL_BUFS", {})
        opool = ctx.enter_context(tc.tile_pool(name="opool", bufs=bufs.get("opool", 4)))
        epool = ctx.enter_context(tc.tile_pool(name="epool", bufs=bufs.get("epool", 2)))
        hpool = ctx.enter_context(tc.tile_pool(name="hpool", bufs=bufs.get("hpool", 2)))
        wpool = ctx.enter_context(tc.tile_pool(name="wpool", bufs=bufs.get("wpool", 3)))
        o16pool = ctx.enter_context(tc.tile_pool(name="o16pool", bufs=bufs.get("o16pool", 3)))
        singles = ctx.enter_context(tc.tile_pool(name="singles", bufs=1))
        pspool = ctx.enter_context(tc.tile_pool(name="ps", bufs=1, space="PSUM"))

        meta_sb = singles.tile([P, NWIN, 2, 2 * SWIN], mybir.dt.uint16)
        sdat_sb = meta_sb[:, :, 0, :].bitcast(f16)
        sidx_sb = meta_sb[:, :, 1, :].bitcast(i16)
        META_CHUNK = 8  # windows per meta DMA chunk
        NCHUNK = NWIN // META_CHUNK
        s_all = singles.tile([P, GP], f16)
        psum_tr = pspool.tile([P, MCOL], f32)

        max_tile = max(tiles)
        for rep in range(repeats):
          g0 = 0
          for ti, gt in enumerate(tiles):
              free_t = gt * C
              if ti < NCHUNK:
                  mlo = ti * META_CHUNK
                  nc.sync.dma_start(
                      out=meta_sb[:, mlo : mlo + META_CHUNK, :, :],
                      in_=meta[:, mlo : mlo + META_CHUNK, :, :],
                  )
              ot = opool.tile([P, max_tile, C], f32, tag="ot", name=f"ot{ti}")[:, :gt, :]
              nc.sync.dma_start(out=ot, in_=o_v[:, g0 : g0 + gt, :])

              s_seg = s_all[:, g0 : g0 + gt]
              if mode != "dma" and gt <= 64:
                  # small (tail) tile: single direct exp + 18-wide segmented
                  # reduce minimizes serial latency after the last input DMA
                  et = epool.tile(
                      [P, 2, max_tile * 9], f16, tag="et", name=f"et{ti}"
                  ).rearrange("p a b -> p (a b)")[:, : gt * C]
                  nc.scalar.activation(
                      et.rearrange("p (g k) -> p g k", k=C), ot, Exp
                  )
                  nc.vector.reduce_sum(
                      s_seg, et.rearrange("p (g k) -> p g k", k=C), axis=X
                  )
              elif mode != "dma":
                  # split-class fold: exp writes class halves 0-8 / 9-17 into
                  # two contiguous fp16 buffers (two 3D-AP instructions); one
                  # step-1 fp16 tensor_add folds them at the DVE 2x_1P rate so
                  # the 1x-rate segmented reduce reads only 9 classes/sample.
                  et = epool.tile(
                      [P, 2, max_tile * 9], f16, tag="et", name=f"et{ti}"
                  )[:, :, : gt * 9]
                  for hh in range(2):
                      nc.scalar.activation(
                          et[:, hh, :].rearrange("p (g k) -> p g k", k=9),
                          ot[:, :, 9 * hh : 9 * hh + 9],
                          Exp,
                      )
                  ht = hpool.tile(
                      [P, max_tile * 9], f16, tag="ht", name=f"ht{ti}"
                  )[:, : gt * 9]
                  nc.vector.tensor_add(ht, et[:, 0, :], et[:, 1, :])
                  nc.vector.reduce_sum(
                      s_seg, ht.rearrange("p (g k) -> p g k", k=9), axis=X
                  )
              if mode != "full":
                  g0 += gt
                  continue

              of = ot.rearrange("p n c -> p (n c)")
              n_blocks = free_t // MCOL
              rem = free_t - n_blocks * MCOL
              # remainder block (if any) second so the first/last matmuls span
              # the full psum partition range (sim group tracking requires
              # start/stop to cover every started zero region)
              order = [0] + ([n_blocks] if rem else []) + list(range(1, n_blocks))

              wt = wpool.tile([P, max_tile * C], f16, tag="wt", name=f"wt{ti}")[:, :free_t]
              w0 = g0 // SWIN
              for w in range(gt // SWIN):
                  nc.gpsimd.local_scatter(
                      wt[:, w * WELEMS : (w + 1) * WELEMS],
                      sdat_sb[:, w0 + w, :],
                      sidx_sb[:, w0 + w, :],
                      channels=P,
                      num_elems=WELEMS,
                      num_idxs=2 * SWIN,
                  )

              o16 = o16pool.tile([P, max_tile * C], f16, tag="o16", name=f"o16_{ti}")[:, :free_t]
              if conv[ti] == "a":
                  nc.scalar.copy(o16, of)
              elif conv[ti] == "t":
                  cut = (free_t * 31 // 100) & ~1
                  nc.scalar.copy(o16[:, :cut], of[:, :cut])
                  nc.vector.tensor_copy(o16[:, cut:], of[:, cut:])
              elif conv[ti] == "h":
                  half = free_t // 2
                  nc.scalar.copy(o16[:, :half], of[:, :half])
                  nc.vector.tensor_copy(o16[:, half:], of[:, half:])
              else:
                  nc.vector.tensor_copy(o16, of)

              for pos, j in enumerate(order):
                  lo = j * MCOL
                  if j < n_blocks:
                      # 128-column lhsT (spills into the next block) enables
                      # fast weight load; extra output rows land in unused
                      # psum partitions.
                      ml = min(P, free_t - lo)
                      nc.tensor.matmul(
                          psum_tr[:ml, :MCOL],
                          o16[:, lo : lo + ml],
                          wt[:, lo : lo + MCOL],
                          start=(ti == 0 and pos == 0),
                          stop=(ti == nt - 1 and pos == len(order) - 1),
                      )
                  else:
                      nc.tensor.matmul(
                          psum_tr[:rem, :rem],
                          o16[:, lo : lo + rem],
                          wt[:, lo : lo + rem],
                          start=False,
                          stop=False,
                      )
              g0 += gt

        if mode == "dma":
            nc.vector.memset(s_all, 1.0)

        tr_sb = singles.tile([P, MCOL], f32)
        if mode == "full":
            nc.vector.tensor_copy(tr_sb, psum_tr)
        else:
            nc.vector.memset(tr_sb, 0.0)
        # two s-out DMAs: the bulk (ready long before the input stream ends)
        # and the last tile's sliver, so the final transfer waits only on the
        # last tiny reduce. All outputs issue from SP after every input DMA.
        g_last = GP - tiles[-1]
        nc.sync.dma_start(out=s_out[:, :g_last], in_=s_all[:, :g_last])
        nc.sync.dma_start(out=s_out[:, g_last:], in_=s_all[:, g_last:])
        nc.sync.dma_start(out=tr_out, in_=tr_sb)

    nc.compile()
    return nc


def _host_prep(outputs, targets, ages):
    """Shard outputs and build per-core scatter index/value tables."""
    t = np.asarray(targets).astype(np.int32)
    age = np.asarray(ages).astype(np.int32)

    b1 = (age > 50) & (age < 60)
    b2 = age == 60
    b3 = (age > 24) & (age < 30)
    b4 = (age > 29) & (age < 35)
    agef = age.astype(np.float32)
    delta = np.where(
        b1,
        (agef - 50) * np.float32(0.05),
        np.where(
            b2,
            np.float32(0.2),
            np.where(
                b3,
                (agef - 20) * np.float32(0.05),
                np.where(b4, (np.float32(39) - agef) * np.float32(0.05), np.float32(0)),
            ),
        ),
    ).astype(np.float32)
    shift = np.where(b1 | b3, 1, np.where(b2 | b4, -1, 0)).astype(np.int32)
    neigh = (t + shift) % C

    a16 = (np.float32(1.0) - delta).astype(np.float16)
    b16 = delta.astype(np.float16)

    # sample s = core*BC + p*GP + win*SWIN + j
    jj = (np.arange(SWIN, dtype=np.int16) * C)[None, None, None, :]
    t_r = t.reshape(NCORES, P, NWIN, SWIN).astype(np.int16)
    n_r = neigh.reshape(NCORES, P, NWIN, SWIN).astype(np.int16)
    d_r = delta.reshape(NCORES, P, NWIN, SWIN)

    # meta[..., 0, :] = fp16 scatter values, meta[..., 1, :] = int16 indices
    meta = np.empty((NCORES, P, NWIN, 2, 2 * SWIN), np.uint16)
    dat = meta[..., 0, :].view(np.float16)
    dat[..., :SWIN] = a16.reshape(NCORES, P, NWIN, SWIN)
    dat[..., SWIN:] = b16.reshape(NCORES, P, NWIN, SWIN)
    idx = meta[..., 1, :].view(np.int16)
    idx[..., :SWIN] = jj + t_r
    idx[..., SWIN:] = np.where(d_r == 0, np.int16(-1), jj + n_r)

    o_sh = np.ascontiguousarray(outputs, dtype=np.float32).reshape(NCORES, BC, C)

    in_maps = [{"o": o_sh[i], "meta": meta[i]} for i in range(NCORES)]
    return in_maps


def kernel(outputs, targets, ages):
    import os

    # NTFF tracing needs an axon profile hook this container lacks; make sure
    # a stray BASS_TRACE can't divert run_bass_kernel_spmd onto that path.
    os.environ["BASS_NEVER_TRACE"] = "1"
    from concourse import bass_utils

    if "nc" not in _CACHE:
        _CACHE["nc"] = _build_bass()
    nc = _CACHE["nc"]

    in_maps = _host_prep(outputs, targets, ages)
    res = bass_utils.run_bass_kernel_spmd(
        nc, in_maps, core_ids=list(range(NCORES))
    )

    total = np.float64(0.0)
    for r in res.results:
        total += np.log(r["s_out"].astype(np.float32)).astype(np.float64).sum()
        total -= np.trace(r["tr_out"][:MCOL, :MCOL].astype(np.float64))
    return np.float32(total / B)
